# revision 9
# baseline (speedup 1.0000x reference)
"""Trainium2 Bass kernel for mean Jaccard index (IoU) over 16 classes.

Computation: argmax over class dim of pred (B,C,H,W) -> hard labels; per-class
intersection/union counts vs target; scores = inter/union (1.0 where union==0);
return mean over classes.

Strategy (data-parallel over 8 NeuronCores, one batch sample per core):
  - Pack the class index c into the 4 low mantissa bits of each fp32 pred
    value: y_c = (bits(pred_c) & ~15) | c.  fp32 ordering is preserved up to
    <=15 ulp perturbations, so max(y_c) carries argmax(pred_c) in its low bits.
  - Per-pixel max over the 16 packed class planes with one strided
    tensor_reduce on DVE; idx = bits(max) & 15.
  - correct = (idx == target); tsel = target - 17*correct  (correct pixels get
    shifted to bins -17..-2, so one histogram of tsel yields the per-class
    intersection counts).
  - Three 16-bin histograms (idx, target, tsel) as bf16 is_equal passes with
    accum_out (exact integer sums in fp32 accumulators, one column per
    (chunk, hist, class)).
  - One PE matmul against a ones vector reduces over the 128 partitions; the
    host sums the per-chunk/per-core count vectors (exact in float64) and does
    the final O(C) IoU arithmetic.
"""

import numpy as np

C = 16  # classes
B = 8  # batch == number of cores
H = W = 512
PIX = H * W  # pixels per core shard
P = 128  # SBUF partitions
NHIST = 3  # idx-hist, target-hist, intersection-hist

_cache = {}


def _build_nc(pix=PIX, f=512, repeat=1):
    import concourse.bacc as bacc
    import concourse.mybir as mybir
    import concourse.tile as tile

    free = pix // P
    nchunk = free // f
    assert nchunk * f == free
    ncol = nchunk * NHIST * C

    nc = bacc.Bacc(target_bir_lowering=False, debug=False)
    pred = nc.dram_tensor("pred", [C, pix], mybir.dt.float32, kind="ExternalInput")
    targ = nc.dram_tensor("target", [pix], mybir.dt.int32, kind="ExternalInput")
    out = nc.dram_tensor("out", [1, ncol], mybir.dt.float32, kind="ExternalOutput")

    pred_r = pred[:].rearrange("c (p f) -> p c f", p=P)  # (128, C, free)
    targ_r = targ[:].rearrange("(p f) -> p f", p=P)  # (128, free)

    Alu = mybir.AluOpType
    with tile.TileContext(nc) as tc:
        with (
            tc.tile_pool(name="predp", bufs=2) as predp,
            tc.tile_pool(name="small", bufs=2) as small,
            tc.tile_pool(name="scr", bufs=4) as scrp,
            tc.tile_pool(name="acc", bufs=1) as accp,
            tc.tile_pool(name="psum", bufs=1, space="PSUM") as psump,
        ):
            accum = accp.tile([P, ncol], mybir.dt.float32)
            ones = accp.tile([P, 1], mybir.dt.float32)
            nc.vector.memset(ones[:], 1.0)

            for k in [kk for _ in range(repeat) for kk in range(nchunk)]:
                y = predp.tile([P, C, f], mybir.dt.float32)
                for c in range(C):
                    nc.sync.dma_start(
                        out=y[:, c, :], in_=pred_r[:, c, k * f : (k + 1) * f]
                    )
                ti = small.tile([P, f], mybir.dt.int32)
                nc.sync.dma_start(out=ti[:], in_=targ_r[:, k * f : (k + 1) * f])

                # pack class index into 4 low mantissa bits (in place)
                yu = y[:].bitcast(mybir.dt.uint32)
                for c in range(C):
                    nc.vector.tensor_scalar(
                        yu[:, c, :],
                        yu[:, c, :],
                        0xFFFFFFF0,
                        c,
                        Alu.bitwise_and,
                        Alu.bitwise_or,
                    )

                # per-pixel max over classes (strided innermost axis)
                m = small.tile([P, f], mybir.dt.float32)
                nc.vector.tensor_reduce(
                    out=m[:],
                    in_=y[:].rearrange("p c f -> p f c"),
                    axis=mybir.AxisListType.X,
                    op=Alu.max,
                )

                # winning class = low 4 bits of the packed max
                idx_i = small.tile([P, f], mybir.dt.uint32)
                nc.vector.tensor_scalar(
                    idx_i[:], m[:].bitcast(mybir.dt.uint32), 15, None, Alu.bitwise_and
                )
                idx_bf = small.tile([P, f], mybir.dt.bfloat16)
                nc.vector.tensor_copy(idx_bf[:], idx_i[:])
                t_bf = small.tile([P, f], mybir.dt.bfloat16)
                nc.vector.tensor_copy(t_bf[:], ti[:])

                # correct = (idx == t); tsel = t - 17*correct
                corr = small.tile([P, f], mybir.dt.bfloat16)
                nc.vector.scalar_tensor_tensor(
                    corr[:], idx_bf[:], 1.0, t_bf[:], Alu.mult, Alu.is_equal
                )
                tsel = small.tile([P, f], mybir.dt.bfloat16)
                nc.vector.scalar_tensor_tensor(
                    tsel[:], corr[:], -17.0, t_bf[:], Alu.mult, Alu.add
                )

                # 3 histograms x 16 bins, exact integer sums into accum columns
                for h, (src, base) in enumerate(
                    [(idx_bf, 0.0), (t_bf, 0.0), (tsel, -17.0)]
                ):
                    for c in range(C):
                        sc = scrp.tile([P, f], mybir.dt.bfloat16, tag="scr")
                        col = (k * NHIST + h) * C + c
                        nc.vector.tensor_scalar(
                            sc[:],
                            src[:],
                            float(c) + base,
                            None,
                            Alu.is_equal,
                            Alu.add,
                            accum_out=accum[:, col : col + 1],
                        )

            # reduce over partitions with a ones-vector matmul
            ps = psump.tile([1, ncol], mybir.dt.float32)
            nc.tensor.matmul(ps[:], ones[:], accum[:], start=True, stop=True)
            outsb = accp.tile([1, ncol], mybir.dt.float32)
            nc.scalar.copy(outsb[:], ps[:])
            nc.sync.dma_start(out=out[:], in_=outsb[:])

    nc.finalize()
    return nc, ncol


def _get_nc(pix=PIX, f=512, repeat=1):
    key = (pix, f, repeat)
    if key not in _cache:
        _cache[key] = _build_nc(pix, f, repeat)
    return _cache[key]


def _decode(outs, nchunk):
    """outs: list of per-core (1, ncol) count vectors -> scores mean (f64)."""
    tot = np.zeros((NHIST, C), dtype=np.float64)
    for o in outs:
        tot += (
            np.asarray(o, dtype=np.float64)
            .reshape(nchunk, NHIST, C)
            .sum(axis=0)
        )
    counts_p, counts_t, inter = tot[0], tot[1], tot[2]
    union = counts_p + counts_t - inter
    scores = np.where(union == 0, 1.0, inter / np.where(union == 0, 1.0, union))
    return scores.mean()


def run(pred, target, trace=False):
    """Returns (result_scalar_f32, BassKernelResults)."""
    from concourse.bass_utils import run_bass_kernel_spmd

    pred = np.asarray(pred, dtype=np.float32)
    target = np.asarray(target, dtype=np.int32)
    assert pred.shape == (B, C, H, W), pred.shape
    assert target.shape == (B, H, W), target.shape

    nc, ncol = _get_nc()
    in_maps = [
        {
            "pred": np.ascontiguousarray(pred[b]).reshape(C, PIX),
            "target": np.ascontiguousarray(target[b]).reshape(PIX),
        }
        for b in range(B)
    ]
    res = run_bass_kernel_spmd(nc, in_maps, core_ids=list(range(B)), trace=trace)
    outs = [r["out"] for r in res.results]
    nchunk = ncol // (NHIST * C)
    mean = _decode(outs, nchunk)
    return np.float32(mean), res


def kernel(pred, target):
    result, _ = run(pred, target)
    return np.asarray(result, dtype=np.float32)
